# revision 35
# baseline (speedup 1.0000x reference)
"""Distributed Trainium2 Bass kernel: masked (upper-triangular) attention.

reference (L=4096, D=1024, fp32):
    Q = x @ Wq + bq ; K = z @ Wk + bk ; V = z @ Wv + bv
    S = Q @ K.T ; S[row > col] = -inf
    out = softmax(S / sqrt(D)) @ V

Strategy (8 NeuronCores, one TRN2 chip, SPMD):
  - Sequence parallel on query rows: core c owns rows [512c, 512c+512).
  - K/V projection sharded over z rows (512/core), AllGathered in bf16
    (K stored transposed [D, L] blocked by shard, V natural [L, D]).
  - Attention computed as S^T tiles (keys on partitions) so the P^T needed by
    the PV matmul comes straight out of the softmax with no transposes.
  - Softmax without max-subtraction (scores here are O(1), exp can't overflow
    in fp32); mask applied multiplicatively after exp, built at runtime from
    an iota constant + a per-core row0 scalar input, keeping one graph valid
    for all cores (SPMD - no per-core control flow).
  - Matmuls in bf16 with fp32 PSUM accumulation (end-to-end rel err ~3e-3).
"""

import math

import numpy as np

import concourse.mybir as mybir
import concourse.tile as tile
from concourse import bacc
from concourse.bass_utils import run_bass_kernel_spmd

F32 = mybir.dt.float32
BF16 = mybir.dt.bfloat16
AF = mybir.ActivationFunctionType
OP = mybir.AluOpType
P = 128
NCORES = 8

L = 4096
D = 1024


def build_graph(Ldim=L, Ddim=D):
    nc = bacc.Bacc("TRN2", target_bir_lowering=False, debug=False, num_devices=NCORES)
    ROWS = Ldim // NCORES        # query rows per core
    MB = ROWS // P               # 128-row m-chunks per core (4)
    ZB = ROWS // P               # z-shard 128-row blocks (4)
    SW = ROWS                    # key-tile width == z-shard width (512)
    JT = SW // P                 # 128-row subtiles per key tile (4)
    NT = NCORES                  # one key tile per shard
    IO = Ddim // P               # contraction chunks (8)
    AO = Ddim // P               # d_attn 128-blocks (8)
    VH = Ddim // 512             # 512-wide value column halves (2)
    HLF = ROWS // 256            # 256-row halves for PV psum pressure (2)
    scale = 1.0 / math.sqrt(Ddim)

    x_ext = nc.declare_dram_parameter("x", [ROWS, Ddim], F32, isOutput=False)
    z_ext = nc.declare_dram_parameter("z", [ROWS, Ddim], F32, isOutput=False)
    wq_ext = nc.declare_dram_parameter("Wq", [Ddim, Ddim], F32, isOutput=False)
    wk_ext = nc.declare_dram_parameter("Wk", [Ddim, Ddim], F32, isOutput=False)
    wv_ext = nc.declare_dram_parameter("Wv", [Ddim, Ddim], F32, isOutput=False)
    bq_ext = nc.declare_dram_parameter("bq", [Ddim], F32, isOutput=False)
    bk_ext = nc.declare_dram_parameter("bk", [Ddim], F32, isOutput=False)
    bv_ext = nc.declare_dram_parameter("bv", [Ddim], F32, isOutput=False)
    row0_ext = nc.declare_dram_parameter("row0", [1], F32, isOutput=False)
    out_ext = nc.declare_dram_parameter("out", [ROWS, Ddim], F32, isOutput=True)

    ident_d = nc.inline_tensor(np.eye(P, dtype=np.float32), name="ident_c")
    ones_d = nc.inline_tensor(np.ones((P, 8), np.float32), name="ones_c")
    # mask keeps where (m - p) + (row0 - SW*t - 128j) <= 0
    njt_np = np.broadcast_to(
        -(float(SW) * np.arange(NT)[:, None] + 128.0 * np.arange(JT)[None, :])
        .astype(np.float32).reshape(1, NT * JT), (P, NT * JT)).copy()
    njt_d = nc.inline_tensor(njt_np, name="njt_c")
    nSWt_d = nc.inline_tensor(
        np.broadcast_to((-float(SW) * np.arange(NT, dtype=np.float32))[None, :], (P, NT)).copy(),
        name="nswt_c")

    with tile.TileContext(nc) as tc:
        with tc.tile_pool(name="const", bufs=1) as constp, \
             tc.tile_pool(name="persist", bufs=1) as persist, \
             tc.tile_pool(name="dram", bufs=1, space="DRAM") as dram:
            ident = constp.tile([P, P], F32)
            nc.sync.dma_start(out=ident[:], in_=ident_d.ap())
            ones_f = constp.tile([P, 8], F32)
            nc.sync.dma_start(out=ones_f[:], in_=ones_d.ap())
            ones8 = constp.tile([P, 8], BF16)
            nc.vector.tensor_copy(ones8[:], ones_f[:])
            bvb = constp.tile([P, Ddim], F32)
            nc.sync.dma_start(out=bvb[:], in_=bv_ext[:].partition_broadcast(P))
            bqs = constp.tile([P, AO], F32)
            nc.sync.dma_start(out=bqs[:], in_=bq_ext[:].rearrange("(ao p) -> p ao", p=P))
            bks = constp.tile([P, AO], F32)
            nc.sync.dma_start(out=bks[:], in_=bk_ext[:].rearrange("(ao p) -> p ao", p=P))
            row0b = constp.tile([P, 1], F32)
            nc.sync.dma_start(out=row0b[:], in_=row0_ext[:].partition_broadcast(P))
            nswt = constp.tile([P, NT], F32)
            nc.sync.dma_start(out=nswt[:], in_=nSWt_d.ap())
            r0t = constp.tile([P, NT], F32)
            nc.vector.tensor_scalar(r0t[:], nswt[:], row0b[:], None, OP.add)

            QT = persist.tile([P, IO, ROWS], BF16)
            KW = AO * ROWS               # flat K width per partition
            VW = ZB * Ddim               # flat V width per partition
            kt_bd = dram.tile([P, KW], BF16)
            v_bds = [dram.tile([P, VW // VH], BF16, name=f"v_bd{vh}") for vh in range(VH)]
            kt_gd = dram.tile([NCORES, P, KW], BF16)
            v_gds = [dram.tile([NCORES, P, VW // VH], BF16, name=f"v_gd{vh}") for vh in range(VH)]

            # ------- Phase 1+2: projections of own shards; K/V AllGathered -------
            with tc.tile_pool(name="inp", bufs=1) as inp, \
                 tc.tile_pool(name="wst", bufs=3) as wst, \
                 tc.tile_pool(name="wkv", bufs=1) as wp, \
                 tc.tile_pool(name="zp", bufs=1) as zp, \
                 tc.tile_pool(name="tpp", bufs=2, space="PSUM") as tpp, \
                 tc.tile_pool(name="pp", bufs=2, space="PSUM") as pp:
                wmup = wst.tile([P, 512], BF16, tag="wm", name="wmup")
                nc.vector.memset(wmup[:], 0.0)
                wpsum = tpp.tile([P, 512], F32, tag="wm", name="wpsum", bufs=1)
                for i in range(24):
                    nc.tensor.matmul(wpsum[:], wmup[:, 0:128], wmup[:], start=True, stop=True)
                zsb = inp.tile([P, ZB, Ddim], F32)
                nc.sync.dma_start(out=zsb[:], in_=z_ext[:].rearrange("(nb p) i -> p nb i", p=P))
                xsb = inp.tile([P, MB, Ddim], F32)
                nc.sync.dma_start(out=xsb[:], in_=x_ext[:].rearrange("(mb p) i -> p mb i", p=P))
                wk = wp.tile([P, IO, Ddim], BF16)
                wv = wp.tile([P, IO, Ddim], BF16)
                wq = wp.tile([P, IO, Ddim], BF16)
                for io in range(IO):
                    ws = wst.tile([P, Ddim], F32, tag="ws", name=f"ws_k_{io}")
                    nc.scalar.dma_start(out=ws[:], in_=wk_ext[io * P:(io + 1) * P, :])
                    nc.vector.tensor_copy(wk[:, io, :], ws[:])
                zT = zp.tile([P, IO, ROWS], BF16)
                for io in range(IO):
                    for nb in range(ZB):
                        tp = tpp.tile([P, P], F32, tag="tp", name=f"tp_{nb}_{io}")
                        nc.tensor.transpose(tp[:], zsb[:, nb, io * P:(io + 1) * P], ident[:])
                        nc.vector.tensor_copy(zT[:, io, nb * P:(nb + 1) * P], tp[:])

                KTs = persist.tile([P, AO, ROWS], BF16)
                for ao in range(AO):
                    kp = pp.tile([P, ROWS], F32, tag="kp", name=f"kp_{ao}")
                    for io in range(IO):
                        nc.tensor.matmul(kp[:], wk[:, io, ao * P:(ao + 1) * P], zT[:, io, :],
                                         start=(io == 0), stop=(io == IO - 1))
                    nc.vector.tensor_scalar(KTs[:, ao, :], kp[:], bks[:, ao:ao + 1], None, OP.add)
                nc.sync.dma_start(out=kt_bd[:], in_=KTs[:])
                nc.gpsimd.collective_compute(
                    "AllGather", OP.bypass, replica_groups=[list(range(NCORES))],
                    ins=[kt_bd[:].opt()], outs=[kt_gd[:].opt()])

                # wv/wq staged after K so their casts stay off the K critical path
                for wi, (eng, wtile, wext) in enumerate((
                        (nc.scalar, wv, wv_ext), (nc.gpsimd, wq, wq_ext))):
                    for io in range(IO):
                        ws = wst.tile([P, Ddim], F32, tag="ws", name=f"ws_{wi}_{io}")
                        eng.dma_start(out=ws[:], in_=wext[io * P:(io + 1) * P, :])
                        nc.vector.tensor_copy(wtile[:, io, :], ws[:])

                # Q^T projection (overlaps the K AllGather)
                xT = zp.tile([P, IO, ROWS], BF16)
                for io in range(IO):
                    for mb in range(MB):
                        tq = tpp.tile([P, P], F32, tag="tp", name=f"tq_{mb}_{io}")
                        nc.tensor.transpose(tq[:], xsb[:, mb, io * P:(io + 1) * P], ident[:])
                        nc.vector.tensor_copy(xT[:, io, mb * P:(mb + 1) * P], tq[:])
                for ao in range(AO):
                    qp = pp.tile([P, ROWS], F32, tag="kp", name=f"qp_{ao}")
                    for io in range(IO):
                        nc.tensor.matmul(qp[:], wq[:, io, ao * P:(ao + 1) * P], xT[:, io, :],
                                         start=(io == 0), stop=(io == IO - 1))
                    # fold the softmax 1/sqrt(D) into Q^T
                    nc.vector.tensor_scalar(QT[:, ao, :], qp[:], bqs[:, ao:ao + 1], float(scale),
                                            OP.add, OP.mult)

                Vs = persist.tile([P, VH, ZB, 512], BF16)
                for nb in range(ZB):
                    vp = pp.tile([P, Ddim], F32, tag="vp", name=f"vp_{nb}", bufs=1)
                    for io in range(IO):
                        for vh in range(VH):
                            nc.tensor.matmul(vp[:, vh * 512:(vh + 1) * 512],
                                             zT[:, io, nb * P:(nb + 1) * P],
                                             wv[:, io, vh * 512:(vh + 1) * 512],
                                             start=(io == 0), stop=(io == IO - 1))
                    for vh in range(VH):
                        nc.vector.tensor_tensor(Vs[:, vh, nb, :], vp[:, vh * 512:(vh + 1) * 512],
                                                bvb[:, vh * 512:(vh + 1) * 512], OP.add)
                for vh in range(VH):
                    nc.sync.dma_start(out=v_bds[vh][:], in_=Vs[:, vh])
                    nc.gpsimd.collective_compute(
                        "AllGather", OP.bypass, replica_groups=[list(range(NCORES))],
                        ins=[v_bds[vh][:].opt()], outs=[v_gds[vh][:].opt()])

            # ---------------- Phase 3: attention ----------------
            acc = persist.tile([P, MB, Ddim], F32)       # PV accumulator (SBUF)
            with tc.tile_pool(name="ktp", bufs=2) as ktp, \
                 tc.tile_pool(name="vtp", bufs=3) as vtp, \
                 tc.tile_pool(name="esp", bufs=8) as esp, \
                 tc.tile_pool(name="recp", bufs=1) as recp:
                # nq[p, t] = 1.0 where tile t is NOT this core's own shard
                nq = constp.tile([P, NT], F32)
                nc.vector.tensor_scalar(nq[:], r0t[:], 0.0, None, OP.not_equal)
                # precompute all masks up front (hides under the AllGather):
                # mk_all[t] keeps where (m-p) + (row0 - SW*t - 128j) <= 0, t != own
                mk_all = persist.tile([P, NT, JT * ROWS], BF16)
                mk_loc = persist.tile([P, JT * ROWS], BF16)
                with tc.tile_pool(name="iop", bufs=1) as iop:
                    iota1 = iop.tile([P, ROWS], F32)
                    nc.gpsimd.iota(iota1[:], pattern=[[1, ROWS]], base=0,
                                   channel_multiplier=-1,
                                   allow_small_or_imprecise_dtypes=True)
                    njt = iop.tile([P, NT * JT], F32)
                    nc.sync.dma_start(out=njt[:], in_=njt_d.ap())
                    r0tj = iop.tile([P, NT * JT], F32)
                    nc.vector.tensor_scalar(r0tj[:], njt[:], row0b[:], None, OP.add)
                    for j in range(JT):
                        nc.vector.tensor_scalar(mk_loc[:, j * ROWS:(j + 1) * ROWS], iota1[:],
                                                float(-128 * j), 0.0, OP.add, OP.is_le)
                    for t in range(NT):
                        for j in range(JT):
                            tj = t * JT + j
                            nc.vector.tensor_scalar(mk_all[:, t, j * ROWS:(j + 1) * ROWS],
                                                    iota1[:], r0tj[:, tj:tj + 1], 0.0,
                                                    OP.add, OP.is_le)
                        nc.vector.tensor_scalar(mk_all[:, t, :], mk_all[:, t, :],
                                                nq[:, t:t + 1], None, OP.mult)

                es_list = []
                lacc = persist.tile([P, MB, 8], F32)

                def attn_pv(tag, es_grp, v_grp, vh, init, pool, lpool):
                    with_l = (vh == 0)
                    for h in range(HLF):
                        pvs = [pool.tile([P, 512], F32, tag=f"pvq{mc}",
                                         name=f"pv{mc}_{tag}_{h}") for mc in range(2)]
                        lts = [lpool.tile([P, 8], F32, tag=f"lt{mc}",
                                          name=f"lt{mc}_{tag}_{h}") for mc in range(2)] if with_l else None
                        np_ = len(es_grp)
                        for ti, (es, v_src) in enumerate(zip(es_grp, v_grp)):
                            for j in range(JT):
                                for mc in range(2):
                                    m0 = h * 256 + mc * P
                                    lhs = es[:, j, m0:m0 + P]
                                    if with_l:
                                        nc.tensor.matmul(lts[mc][:], lhs, ones8[:],
                                                         start=(ti == 0 and j == 0),
                                                         stop=(ti == np_ - 1 and j == JT - 1))
                                    nc.tensor.matmul(pvs[mc][:], lhs, v_src[:, j, :],
                                                     start=(ti == 0 and j == 0),
                                                     stop=(ti == np_ - 1 and j == JT - 1))
                        for mc in range(2):
                            gmc = 2 * h + mc
                            vsl = slice(vh * 512, (vh + 1) * 512)
                            if init:
                                nc.vector.tensor_copy(acc[:, gmc, vsl], pvs[mc][:])
                                if with_l:
                                    nc.vector.tensor_copy(lacc[:, gmc, :], lts[mc][:])
                            else:
                                nc.vector.tensor_tensor(acc[:, gmc, vsl], acc[:, gmc, vsl],
                                                        pvs[mc][:], OP.add)
                                if with_l:
                                    nc.vector.tensor_tensor(lacc[:, gmc, :], lacc[:, gmc, :],
                                                            lts[mc][:], OP.add)

                with tc.tile_pool(name="spp", bufs=2, space="PSUM") as spp, \
                     tc.tile_pool(name="lpp", bufs=1, space="PSUM") as lpp, \
                     tc.tile_pool(name="pvg", bufs=2, space="PSUM") as pvg:

                    def attn_s(tag, kt_src, mk_ap, es_tag="es"):
                        es = esp.tile([P, JT, ROWS], BF16, tag=es_tag, name=f"es_{tag}")
                        for j in range(JT):
                            sp = spp.tile([P, ROWS], F32, tag="sp", name=f"sp_{tag}_{j}")
                            for io in range(IO):
                                nc.tensor.matmul(sp[:], kt_src[:, io, j * P:(j + 1) * P],
                                                 QT[:, io, :], start=(io == 0),
                                                 stop=(io == IO - 1))
                            nc.scalar.activation(es[:, j, :], sp[:], AF.Exp)
                        nc.vector.tensor_tensor(es[:].rearrange("p j m -> p (j m)"),
                                                es[:].rearrange("p j m -> p (j m)"),
                                                mk_ap, OP.mult)
                        return es

                    # local pre-pass on this core's own shard - overlaps the CCs
                    es_l = attn_s("loc", KTs, mk_loc[:], es_tag="esl")
                    for vh in range(VH):
                        attn_pv(f"loc{vh}", [es_l], [Vs[:, vh]], vh, init=True,
                                pool=pvg, lpool=lpp)

                    # S pass for all gathered key tiles (overlaps the V AllGathers)
                    for t in range(NT):
                        ktt = ktp.tile([P, IO, SW], BF16, tag="ktt", name=f"ktt_{t}")
                        nc.sync.dma_start(out=ktt[:], in_=kt_gd[t])
                        es_list.append(attn_s(f"g{t}", ktt, mk_all[:, t, :]))

                    # PV passes per value-half over tile quads; vh0 (plus all
                    # row-sums) hides under the second V AllGather
                    QUAD = 4
                    for vh in range(VH):
                        for pi in range(NT // QUAD):
                            ts_ = list(range(QUAD * pi, QUAD * (pi + 1)))
                            vquad = []
                            for t in ts_:
                                vtt = vtp.tile([P, JT, 512], BF16, tag="vtt",
                                               name=f"vtt_{vh}_{t}")
                                nc.sync.dma_start(out=vtt[:], in_=v_gds[vh][t])
                                vquad.append(vtt)
                            attn_pv(f"p{vh}_{pi}", [es_list[t] for t in ts_], vquad, vh,
                                    init=False, pool=pvg, lpool=lpp)

                # normalize and write out
                for gmc in range(MB):
                    rec = recp.tile([P, 1], F32, tag=f"rec{gmc}", name=f"rec_{gmc}")
                    nc.vector.reciprocal(rec[:], lacc[:, gmc, 0:1])
                    nc.vector.tensor_scalar(acc[:, gmc, :], acc[:, gmc, :], rec[:],
                                            None, OP.mult)
                nc.sync.dma_start(out=out_ext[:].rearrange("(mb p) v -> p mb v", p=P),
                                  in_=acc[:])
    nc.compile()
    return nc


_GRAPH_CACHE = {}


def _get_graph(Ldim=L, Ddim=D):
    key = (Ldim, Ddim)
    if key not in _GRAPH_CACHE:
        _GRAPH_CACHE[key] = build_graph(Ldim, Ddim)
    return _GRAPH_CACHE[key]


def kernel(x, z, Wq, bq, Wk, bk, Wv, bv):
    x = np.ascontiguousarray(np.asarray(x, dtype=np.float32))
    z = np.ascontiguousarray(np.asarray(z, dtype=np.float32))
    Ldim, Ddim = x.shape
    nc = _get_graph(Ldim, Ddim)
    ROWS = Ldim // NCORES
    common = {
        "Wq": np.ascontiguousarray(np.asarray(Wq, np.float32)),
        "bq": np.ascontiguousarray(np.asarray(bq, np.float32)),
        "Wk": np.ascontiguousarray(np.asarray(Wk, np.float32)),
        "bk": np.ascontiguousarray(np.asarray(bk, np.float32)),
        "Wv": np.ascontiguousarray(np.asarray(Wv, np.float32)),
        "bv": np.ascontiguousarray(np.asarray(bv, np.float32)),
    }
    in_maps = []
    for c in range(NCORES):
        m = dict(common)
        m["x"] = x[ROWS * c:ROWS * (c + 1)]
        m["z"] = z[ROWS * c:ROWS * (c + 1)]
        m["row0"] = np.array([ROWS * c], dtype=np.float32)
        in_maps.append(m)
    try:
        res = run_bass_kernel_spmd(nc, in_maps, core_ids=list(range(NCORES)))
    except Exception:
        # transient NRT device hiccups have been observed; one retry
        res = run_bass_kernel_spmd(nc, in_maps, core_ids=list(range(NCORES)))
    out = np.empty((Ldim, Ddim), dtype=np.float32)
    for c in range(NCORES):
        out[ROWS * c:ROWS * (c + 1)] = res.results[c]["out"]
    return out
